# revision 1
# baseline (speedup 1.0000x reference)
"""GPT2 attention (B=2,S=2048,D=1024,H=16,hd=64, no causal mask) on 8 trn2 cores.

Sharding: core c handles batch b=c//4 and head-group g=c%4 (4 heads).
w_attn columns are split per head group (Q scaled by 1/sqrt(hd) on host);
w_proj rows split per head group; host sums the 4 partial c_proj outputs
per batch (the "all-reduce").

Per-core dataflow (matmuls in float32r, 1 cyc/row at N>=512; every tile a
matmul consumes is written as float32r by its producer so walrus' rounding
check passes):
  hid [2048,1024] --PE transpose--> hidT [1024,2048]
  qkvT[768,2048] = w_slice.T @ hidT   (feature-major Q^T,K^T,V^T, 2 heads/tile)
  V^T --PE transpose--> vaug [k,65] tiles (col 64 = ones for denominator)
  per (head, 512-wide q chunk):
    S^T[k,q] tiles = K^T_tile.T @ Q^T  -> DVE copy to SBUF block [128, 4096]
    one ACT exp per block (amortizes ACT fixed cost; no max-subtraction:
    scores are O(1) so exp is numerically safe)
    O_u^T[65,512] = sum_k vaug.T @ E   (row 64 = softmax denominator)
    obar_h = O_u^T[0:64] * broadcast(1/denom)  (ones-matmul broadcast + DVE mul)
  out[q,1024] = sum_h obar_h.T @ wp_h  (K=64 accumulation, 4 heads)
"""

import sys

import numpy as np

if "/opt/trn_rl_repo" not in sys.path:
    sys.path.insert(0, "/opt/trn_rl_repo")

S = 2048
D = 1024
P = 128
NH = 4  # heads per core
HD = 64
N_CORES = 8

_CACHE = {}


def _build_program():
    import concourse.mybir as mybir
    from concourse import bacc
    from concourse.masks import make_identity
    from concourse.tile import TileContext

    f32r = mybir.dt.float32r
    f32 = mybir.dt.float32
    AF = mybir.ActivationFunctionType
    ALU = mybir.AluOpType

    nc = bacc.Bacc(None, target_bir_lowering=False, debug=False)
    hid = nc.declare_dram_parameter("hid", [S, D], f32r, isOutput=False)
    wqkv = nc.declare_dram_parameter("wqkv", [D, 3 * NH * HD], f32r, isOutput=False)
    wp = nc.declare_dram_parameter("wp", [NH * HD, D], f32r, isOutput=False)
    out = nc.declare_dram_parameter("out", [S, D], f32, isOutput=True)

    with TileContext(nc) as tc:
        with tc.tile_pool(name="const", bufs=1) as constp:
            ident_f = constp.tile([P, P], f32)
            make_identity(nc, ident_f)
            ident = constp.tile([P, P], f32r)
            nc.vector.tensor_copy(ident[:], ident_f[:])
            ones_f = constp.tile([P, HD], f32)
            nc.gpsimd.memset(ones_f[:], 1.0)
            ones_t = constp.tile([P, HD], f32r)
            nc.vector.tensor_copy(ones_t[:], ones_f[:])

            qkvT = [constp.tile([P, S], f32r, name=f"qkvT{i}") for i in range(6)]
            vaug = constp.tile([P, NH * 16 * 65], f32r)

            # ---------------- Stage A: hidT + QKV ----------------
            with tc.tile_pool(name="hidT_pool", bufs=1) as hidTp, \
                 tc.tile_pool(name="stageA", bufs=3) as sA, \
                 tc.tile_pool(name="w_pool", bufs=1) as wpool, \
                 tc.tile_pool(name="tpsum", bufs=3, space="PSUM") as tpsum, \
                 tc.tile_pool(name="qpsum", bufs=3, space="PSUM") as qpsum:
                hidT = [hidTp.tile([P, S], f32r, name=f"hidT{i}") for i in range(8)]
                w_sb = [wpool.tile([P, 768], f32r, name=f"w{i}") for i in range(8)]
                for i in range(8):
                    nc.sync.dma_start(out=w_sb[i][:], in_=wqkv[i * P : (i + 1) * P, :])
                for st in range(16):
                    ht = sA.tile([P, D], f32r, tag="hidload")
                    nc.sync.dma_start(out=ht[:], in_=hid[st * P : (st + 1) * P, :])
                    for dt_ in range(8):
                        tp = tpsum.tile([P, P], f32r, tag="tp")
                        nc.tensor.transpose(
                            tp[:], ht[:, dt_ * P : (dt_ + 1) * P], ident[:]
                        )
                        nc.vector.tensor_copy(
                            hidT[dt_][:, st * P : (st + 1) * P], tp[:]
                        )
                for ct in range(6):
                    for qc in range(4):
                        ps = qpsum.tile([P, 512], f32, tag="qkvps")
                        for dt_ in range(8):
                            nc.tensor.matmul(
                                ps[:],
                                lhsT=w_sb[dt_][:, ct * P : (ct + 1) * P],
                                rhs=hidT[dt_][:, qc * 512 : (qc + 1) * 512],
                                start=(dt_ == 0),
                                stop=(dt_ == 7),
                            )
                        nc.vector.tensor_copy(
                            qkvT[ct][:, qc * 512 : (qc + 1) * 512], ps[:]
                        )
                # V seq-major (transpose V^T) into vaug; col 64 of each 65 = ones
                for h in range(NH):
                    par = HD * (h % 2)
                    vsrc = qkvT[4 + h // 2]
                    for kt in range(16):
                        vp = tpsum.tile([P, P], f32r, tag="tp")
                        nc.tensor.transpose(
                            vp[:, :HD],
                            vsrc[par : par + HD, kt * P : (kt + 1) * P],
                            ident[par : par + HD, par : par + HD],
                        )
                        base = (h * 16 + kt) * 65
                        nc.vector.tensor_copy(vaug[:, base : base + HD], vp[:, :HD])
                        nc.vector.tensor_copy(
                            vaug[:, base + HD : base + 65], ones_f[:, 0:1]
                        )

            # ---------------- Stages B+C ----------------
            with tc.tile_pool(name="persistBC", bufs=1) as perBC:
                obar = [perBC.tile([HD, S], f32r, name=f"obar{i}") for i in range(NH)]
                wp_sb = [perBC.tile([HD, D], f32r, name=f"wp{i}") for i in range(NH)]
                for h in range(NH):
                    nc.sync.dma_start(
                        out=wp_sb[h][:], in_=wp[h * HD : (h + 1) * HD, :]
                    )

                with tc.tile_pool(name="sblk", bufs=3) as sblk, \
                     tc.tile_pool(name="npool", bufs=3) as npool, \
                     tc.tile_pool(name="spsum", bufs=2, space="PSUM") as spsum, \
                     tc.tile_pool(name="opsum", bufs=1, space="PSUM") as opsum, \
                     tc.tile_pool(name="rpsum", bufs=1, space="PSUM") as rpsum:
                    for h in range(NH):
                        par = HD * (h % 2)
                        qT = qkvT[0 + h // 2]
                        kT = qkvT[2 + h // 2]
                        for qc in range(2):
                            q0 = qc * 1024
                            op = opsum.tile([65, 1024], f32, tag="op")
                            for kt in range(16):
                                sp = spsum.tile([P, 1024], f32, tag="sp")
                                for u in range(2):
                                    nc.tensor.matmul(
                                        sp[:, u * 512 : (u + 1) * 512],
                                        lhsT=kT[par : par + HD, kt * P : (kt + 1) * P],
                                        rhs=qT[par : par + HD, q0 + u * 512 : q0 + (u + 1) * 512],
                                        start=True,
                                        stop=True,
                                    )
                                eb = sblk.tile([P, 1024], f32r, tag="sb")
                                nc.scalar.activation(eb[:], sp[:], AF.Exp)
                                base = (h * 16 + kt) * 65
                                for u in range(2):
                                    nc.tensor.matmul(
                                        op[:, u * 512 : (u + 1) * 512],
                                        lhsT=vaug[:, base : base + 65],
                                        rhs=eb[:, u * 512 : (u + 1) * 512],
                                        start=(kt == 0),
                                        stop=(kt == 15),
                                    )
                            rec = npool.tile([P, 1024], f32r, tag="rec")
                            with nc.allow_low_precision(
                                reason="f32r recip of softmax denom"
                            ):
                                nc.vector.reciprocal(rec[64:65, :], op[64:65, :])
                            rb = rpsum.tile([HD, 1024], f32, tag="rb")
                            for u in range(2):
                                nc.tensor.matmul(
                                    rb[:, u * 512 : (u + 1) * 512],
                                    lhsT=ones_t[64:65, :],
                                    rhs=rec[64:65, u * 512 : (u + 1) * 512],
                                    start=True, stop=True,
                                )
                            ou_sb = npool.tile([HD, 1024], f32r, tag="ou")
                            nc.vector.tensor_copy(ou_sb[:], op[0:HD, :])
                            rb_sb = npool.tile([HD, 1024], f32r, tag="rbs")
                            nc.vector.tensor_copy(rb_sb[:], rb[:])
                            with nc.allow_low_precision(
                                reason="softmax normalize in f32r"
                            ):
                                nc.vector.tensor_tensor(
                                    out=obar[h][:, q0 : q0 + 1024],
                                    in0=ou_sb[:],
                                    in1=rb_sb[:],
                                    op=ALU.mult,
                                )

                # ---------------- Stage C: projection ----------------
                with tc.tile_pool(name="outp", bufs=4) as outp, \
                     tc.tile_pool(name="ppsum", bufs=4, space="PSUM") as ppsum:
                    for qt in range(16):
                        ot = outp.tile([P, D], f32, tag="ot")
                        for ec in range(2):
                            pp = ppsum.tile([P, 512], f32, tag="pp")
                            for h in range(NH):
                                nc.tensor.matmul(
                                    pp[:],
                                    lhsT=obar[h][:, qt * P : (qt + 1) * P],
                                    rhs=wp_sb[h][:, ec * 512 : (ec + 1) * 512],
                                    start=(h == 0),
                                    stop=(h == NH - 1),
                                )
                            nc.vector.tensor_copy(
                                ot[:, ec * 512 : (ec + 1) * 512], pp[:]
                            )
                        nc.sync.dma_start(
                            out=out[qt * P : (qt + 1) * P, :], in_=ot[:]
                        )

    nc.compile()
    return nc


def _get_nc():
    if "nc" not in _CACHE:
        _CACHE["nc"] = _build_program()
    return _CACHE["nc"]


def _shard_inputs(hidden_states, w_attn, w_proj):
    scale = 1.0 / np.sqrt(np.float32(HD))
    in_maps = []
    for c in range(N_CORES):
        b, g = divmod(c, 4)
        cs = slice(g * NH * HD, (g + 1) * NH * HD)
        wq = w_attn[:, 0:D][:, cs] * scale
        wk = w_attn[:, D : 2 * D][:, cs]
        wv = w_attn[:, 2 * D : 3 * D][:, cs]
        in_maps.append(
            {
                "hid": np.ascontiguousarray(hidden_states[b], dtype=np.float32),
                "wqkv": np.ascontiguousarray(
                    np.concatenate([wq, wk, wv], axis=1), dtype=np.float32
                ),
                "wp": np.ascontiguousarray(w_proj[cs, :], dtype=np.float32),
            }
        )
    return in_maps


def run(hidden_states, w_attn, w_proj, trace=False):
    from concourse.bass_utils import run_bass_kernel_spmd

    nc = _get_nc()
    in_maps = _shard_inputs(hidden_states, w_attn, w_proj)
    res = run_bass_kernel_spmd(nc, in_maps, list(range(N_CORES)), trace=trace)
    parts = [res.results[c]["out"] for c in range(N_CORES)]
    out = np.stack(
        [
            parts[0] + parts[1] + parts[2] + parts[3],
            parts[4] + parts[5] + parts[6] + parts[7],
        ]
    ).astype(np.float32)
    return out, res


def kernel(hidden_states, w_attn, w_proj):
    out, _ = run(
        np.asarray(hidden_states), np.asarray(w_attn), np.asarray(w_proj)
    )
    return out



# revision 8
# speedup vs baseline: 1.4839x; 1.4839x over previous
"""GPT2 attention (B=2,S=2048,D=1024,H=16,hd=64, no causal mask) on 8 trn2 cores.

Sharding: core c handles batch b=c//4 and head-group g=c%4 (4 heads).
w_attn columns are split per head group (Q scaled by 1/sqrt(hd) on host);
w_proj rows split per head group; host sums the 4 partial c_proj outputs
per batch (the "all-reduce").

Per-core dataflow (matmuls in float32r, 1 cyc/row at N>=512; every tile a
matmul consumes is written as float32r by its producer so walrus' rounding
check passes):
  hid [2048,1024] --PE transpose--> hidT [1024,2048]
  qkvT[768,2048] = w_slice.T @ hidT   (feature-major Q^T,K^T,V^T, 2 heads/tile)
  V^T --PE transpose--> vaug [k,65] tiles (col 64 = ones for denominator)
  per (head, 512-wide q chunk):
    S^T[k,q] tiles = K^T_tile.T @ Q^T  -> DVE copy to SBUF block [128, 4096]
    one ACT exp per block (amortizes ACT fixed cost; no max-subtraction:
    scores are O(1) so exp is numerically safe)
    O_u^T[65,512] = sum_k vaug.T @ E   (row 64 = softmax denominator)
    obar_h = O_u^T[0:64] * broadcast(1/denom)  (ones-matmul broadcast + DVE mul)
  out[q,1024] = sum_h obar_h.T @ wp_h  (K=64 accumulation, 4 heads)
"""

import sys

import numpy as np

if "/opt/trn_rl_repo" not in sys.path:
    sys.path.insert(0, "/opt/trn_rl_repo")

S = 2048
D = 1024
P = 128
NH = 4  # heads per core
HD = 64
N_CORES = 8

_CACHE = {}


def _build_program():
    import concourse.mybir as mybir
    from concourse import bacc
    from concourse.masks import make_identity
    from concourse.tile import TileContext

    f32r = mybir.dt.float32r
    f32 = mybir.dt.float32
    AF = mybir.ActivationFunctionType
    ALU = mybir.AluOpType

    nc = bacc.Bacc(None, target_bir_lowering=False, debug=False)
    hid = nc.declare_dram_parameter("hid", [S, D], f32r, isOutput=False)
    wqkv = nc.declare_dram_parameter("wqkv", [D, 3 * NH * HD], f32r, isOutput=False)
    wp = nc.declare_dram_parameter("wp", [NH * HD, D], f32r, isOutput=False)
    out = nc.declare_dram_parameter("out", [S, D], f32, isOutput=True)

    with TileContext(nc) as tc:
        with tc.tile_pool(name="const", bufs=1) as constp:
            ident_f = constp.tile([P, P], f32)
            make_identity(nc, ident_f)
            ident = constp.tile([P, P], f32r)
            nc.vector.tensor_copy(ident[:], ident_f[:])
            ones_f = constp.tile([P, HD], f32)
            nc.gpsimd.memset(ones_f[:], 1.0)
            ones_t = constp.tile([P, HD], f32r)
            nc.vector.tensor_copy(ones_t[:], ones_f[:])

            qkvT = [constp.tile([P, S], f32r, name=f"qkvT{i}") for i in range(6)]
            vaug = constp.tile([P, NH * 16 * 65], f32r)

            # ---------------- Stage A: hidT + QKV ----------------
            with tc.tile_pool(name="hidT_pool", bufs=1) as hidTp, \
                 tc.tile_pool(name="stageA", bufs=3) as sA, \
                 tc.tile_pool(name="w_pool", bufs=1) as wpool, \
                 tc.tile_pool(name="tpsum", bufs=3, space="PSUM") as tpsum, \
                 tc.tile_pool(name="qpsum", bufs=3, space="PSUM") as qpsum:
                hidT = [hidTp.tile([P, S], f32r, name=f"hidT{i}") for i in range(8)]
                w_sb = [wpool.tile([P, 768], f32r, name=f"w{i}") for i in range(8)]
                for i in range(8):
                    nc.sync.dma_start(out=w_sb[i][:], in_=wqkv[i * P : (i + 1) * P, :])
                for st in range(16):
                    ht = sA.tile([P, D], f32r, tag="hidload")
                    nc.sync.dma_start(out=ht[:], in_=hid[st * P : (st + 1) * P, :])
                    for dt_ in range(8):
                        tp = tpsum.tile([P, P], f32r, tag="tp")
                        nc.tensor.transpose(
                            tp[:], ht[:, dt_ * P : (dt_ + 1) * P], ident[:]
                        )
                        nc.vector.tensor_copy(
                            hidT[dt_][:, st * P : (st + 1) * P], tp[:]
                        )
                for ct in range(6):
                    for qc in range(4):
                        ps = qpsum.tile([P, 512], f32, tag="qkvps")
                        for dt_ in range(8):
                            nc.tensor.matmul(
                                ps[:],
                                lhsT=w_sb[dt_][:, ct * P : (ct + 1) * P],
                                rhs=hidT[dt_][:, qc * 512 : (qc + 1) * 512],
                                start=(dt_ == 0),
                                stop=(dt_ == 7),
                            )
                        nc.vector.tensor_copy(
                            qkvT[ct][:, qc * 512 : (qc + 1) * 512], ps[:]
                        )
                # V seq-major (transpose V^T) into vaug; col 64 of each 65 = ones
                for h in range(NH):
                    par = HD * (h % 2)
                    vsrc = qkvT[4 + h // 2]
                    for kt in range(16):
                        vp = tpsum.tile([P, P], f32r, tag="tp")
                        nc.tensor.transpose(
                            vp[:, :HD],
                            vsrc[par : par + HD, kt * P : (kt + 1) * P],
                            ident[par : par + HD, par : par + HD],
                        )
                        base = (h * 16 + kt) * 65
                        nc.vector.tensor_copy(vaug[:, base : base + HD], vp[:, :HD])
                        nc.vector.tensor_copy(
                            vaug[:, base + HD : base + 65], ones_f[:, 0:1]
                        )

            # ---------------- Stages B+C ----------------
            with tc.tile_pool(name="persistBC", bufs=1) as perBC:
                obar = [perBC.tile([HD, S], f32r, name=f"obar{i}") for i in range(NH)]
                wp_sb = [perBC.tile([HD, D], f32r, name=f"wp{i}") for i in range(NH)]
                for h in range(NH):
                    nc.sync.dma_start(
                        out=wp_sb[h][:], in_=wp[h * HD : (h + 1) * HD, :]
                    )

                with tc.tile_pool(name="sblk", bufs=3) as sblk, \
                     tc.tile_pool(name="npool", bufs=3) as npool, \
                     tc.tile_pool(name="spsum", bufs=2, space="PSUM") as spsum, \
                     tc.tile_pool(name="opsum", bufs=1, space="PSUM") as opsum, \
                     tc.tile_pool(name="rpsum", bufs=1, space="PSUM") as rpsum:
                    for h in range(NH):
                        par = HD * (h % 2)
                        qT = qkvT[0 + h // 2]
                        kT = qkvT[2 + h // 2]
                        for qc in range(2):
                            q0 = qc * 1024
                            op = opsum.tile([65, 1024], f32, tag="op")
                            for kt in range(16):
                                sp = spsum.tile([P, 1024], f32, tag="sp")
                                for u in range(2):
                                    nc.tensor.matmul(
                                        sp[:, u * 512 : (u + 1) * 512],
                                        lhsT=kT[par : par + HD, kt * P : (kt + 1) * P],
                                        rhs=qT[par : par + HD, q0 + u * 512 : q0 + (u + 1) * 512],
                                        start=True,
                                        stop=True,
                                    )
                                eb = sblk.tile([P, 1024], f32r, tag="sb")
                                nc.scalar.activation(eb[:], sp[:], AF.Exp)
                                base = (h * 16 + kt) * 65
                                for u in range(2):
                                    nc.tensor.matmul(
                                        op[:, u * 512 : (u + 1) * 512],
                                        lhsT=vaug[:, base : base + 65],
                                        rhs=eb[:, u * 512 : (u + 1) * 512],
                                        start=(kt == 0),
                                        stop=(kt == 15),
                                    )
                            rec = npool.tile([P, 1024], f32r, tag="rec")
                            with nc.allow_low_precision(
                                reason="f32r recip of softmax denom"
                            ):
                                nc.vector.reciprocal(rec[64:65, :], op[64:65, :])
                            rb = rpsum.tile([HD, 1024], f32, tag="rb")
                            for u in range(2):
                                nc.tensor.matmul(
                                    rb[:, u * 512 : (u + 1) * 512],
                                    lhsT=ones_t[64:65, :],
                                    rhs=rec[64:65, u * 512 : (u + 1) * 512],
                                    start=True, stop=True,
                                )
                            ou_sb = npool.tile([HD, 1024], f32r, tag="ou")
                            nc.vector.tensor_copy(ou_sb[:], op[0:HD, :])
                            rb_sb = npool.tile([HD, 1024], f32r, tag="rbs")
                            nc.vector.tensor_copy(rb_sb[:], rb[:])
                            with nc.allow_low_precision(
                                reason="softmax normalize in f32r"
                            ):
                                nc.vector.tensor_tensor(
                                    out=obar[h][:, q0 : q0 + 1024],
                                    in0=ou_sb[:],
                                    in1=rb_sb[:],
                                    op=ALU.mult,
                                )

                # ---------------- Stage C: projection ----------------
                with tc.tile_pool(name="outp", bufs=4) as outp, \
                     tc.tile_pool(name="ppsum", bufs=4, space="PSUM") as ppsum:
                    for qt in range(16):
                        ot = outp.tile([P, D], f32, tag="ot")
                        for ec in range(2):
                            pp = ppsum.tile([P, 512], f32, tag="pp")
                            for h in range(NH):
                                nc.tensor.matmul(
                                    pp[:],
                                    lhsT=obar[h][:, qt * P : (qt + 1) * P],
                                    rhs=wp_sb[h][:, ec * 512 : (ec + 1) * 512],
                                    start=(h == 0),
                                    stop=(h == NH - 1),
                                )
                            nc.vector.tensor_copy(
                                ot[:, ec * 512 : (ec + 1) * 512], pp[:]
                            )
                        nc.sync.dma_start(
                            out=out[qt * P : (qt + 1) * P, :], in_=ot[:]
                        )

    nc.compile()
    return nc


def _get_nc():
    if "nc" not in _CACHE:
        _CACHE["nc"] = _build_program()
    return _CACHE["nc"]


def _shard_inputs(hidden_states, w_attn, w_proj):
    scale = 1.0 / np.sqrt(np.float32(HD))
    in_maps = []
    for c in range(N_CORES):
        b, g = divmod(c, 4)
        cs = slice(g * NH * HD, (g + 1) * NH * HD)
        wq = w_attn[:, 0:D][:, cs] * scale
        wk = w_attn[:, D : 2 * D][:, cs]
        wv = w_attn[:, 2 * D : 3 * D][:, cs]
        in_maps.append(
            {
                "hid": np.ascontiguousarray(hidden_states[b], dtype=np.float32),
                "wqkv": np.ascontiguousarray(
                    np.concatenate([wq, wk, wv], axis=1), dtype=np.float32
                ),
                "wp": np.ascontiguousarray(w_proj[cs, :], dtype=np.float32),
            }
        )
    return in_maps


def run(hidden_states, w_attn, w_proj, trace=False):
    from concourse.bass_utils import run_bass_kernel_spmd

    nc = _get_nc()
    in_maps = _shard_inputs(hidden_states, w_attn, w_proj)
    res = run_bass_kernel_spmd(nc, in_maps, list(range(N_CORES)), trace=trace)
    parts = [res.results[c]["out"] for c in range(N_CORES)]
    out = np.stack(
        [
            parts[0] + parts[1] + parts[2] + parts[3],
            parts[4] + parts[5] + parts[6] + parts[7],
        ]
    ).astype(np.float32)
    return out, res


def kernel(hidden_states, w_attn, w_proj):
    out, _ = run(
        np.asarray(hidden_states), np.asarray(w_attn), np.asarray(w_proj)
    )
    return out


# revision 9
# speedup vs baseline: 1.4984x; 1.0098x over previous
"""GPT2 attention (B=2,S=2048,D=1024,H=16,hd=64, no causal mask) on 8 trn2 cores.

Sharding: core c handles batch b=c//4 and head-group g=c%4 (4 heads).
w_attn columns split per head group (Q pre-scaled by 1/sqrt(hd) on host);
w_proj rows split per head group; host sums the 4 partial c_proj outputs
per batch (the "all-reduce"). All device compute in bf16 with fp32 PSUM
accumulation; I/O tensors bf16 (host casts / upcasts).

Per-core dataflow:
  hidT[1024,2048] <- 8x DMA-transpose loads straight from DRAM (no PE work)
  qkvT[512,2048]  = w.T @ hidT  (feature-major Q^T,K^T; head pair hp
                    occupies partitions 0-63 / 64-127 of its row-tile)
  vaug[sk, 4*16*65] <- V computed seq-major directly: per sk-tile,
                    hidT-tile.T @ wv -> [sk,256] psum -> 4 DVE copies; col 64
                    of each 65-block is a pre-memset ones column (denominator)
  per (head-pair hp, 512-col q chunk):
    for kt in 16:  S[128,1024] psum = [S_A | S_B] via two matmuls whose
                   lhsT/rhs sit at base partitions 0/64 (concurrent row-tiles)
      E[128,1024] bf16 = one ACT exp over both banks (no max-subtraction:
                   scores are O(1) so exp is numerically safe)
      O_h[65,512] += vaug_h.T @ E_h  (row 64 = softmax denominator)
    normalize: reciprocal_approx_fast on denom row -> ones-matmul broadcast
      -> DVE mult -> obar (bf16)
    head B obar is DMA partition-shifted to rows 64-127 of the pair tile
  out[s,1024] = sum_hp obar_hp.T @ wp_hp  (K=128 per pair, 2 accumulating
                matmuls per psum tile)
"""

import sys

import numpy as np

if "/opt/trn_rl_repo" not in sys.path:
    sys.path.insert(0, "/opt/trn_rl_repo")

S = 2048
D = 1024
P = 128
NH = 4  # heads per core
HD = 64
N_CORES = 8

_CACHE = {}

OUT_NAMES = ["out"]


def _build_program():
    import concourse.mybir as mybir
    from concourse import bacc
    from concourse.tile import TileContext

    bf16 = mybir.dt.bfloat16
    f32 = mybir.dt.float32
    f32r = mybir.dt.float32r
    AF = mybir.ActivationFunctionType
    ALU = mybir.AluOpType

    nc = bacc.Bacc(None, target_bir_lowering=False, debug=False)
    hid = nc.declare_dram_parameter("hid", [S, D], bf16, isOutput=False)
    wqkv = nc.declare_dram_parameter("wqkv", [D, 3 * NH * HD], bf16, isOutput=False)
    wp = nc.declare_dram_parameter("wp", [NH * HD, D], bf16, isOutput=False)
    out = nc.declare_dram_parameter("out", [S, D], bf16, isOutput=True)

    with TileContext(nc) as tc:
        with tc.tile_pool(name="persist", bufs=1) as per:
            ones_f = per.tile([P, HD], f32)
            nc.gpsimd.memset(ones_f[:], 1.0)
            ones_r = per.tile([P, HD], f32r)
            nc.vector.tensor_copy(ones_r[:], ones_f[:])

            # vaug: per (head, kt): 65 cols = [V(64) | ones]; ones pre-set.
            vaug = per.tile([P, NH * 16 * 65], bf16)
            nc.gpsimd.memset(vaug[:], 1.0)

            qkvT = [per.tile([P, S], bf16, name=f"qkvT{i}") for i in range(4)]
            obar = [per.tile([P, S], bf16, name=f"obar{i}") for i in range(2)]
            wp_sb = [per.tile([P, D], bf16, name=f"wp{i}") for i in range(2)]
            for i in range(2):
                nc.sync.dma_start(out=wp_sb[i][:], in_=wp[i * P : (i + 1) * P, :])

            # ------------- Stage A/B: hidT (DMA transpose) + QKV -------------
            with tc.tile_pool(name="hidT_pool", bufs=1) as hidTp, \
                 tc.tile_pool(name="w_pool", bufs=1) as wpool, \
                 tc.tile_pool(name="tmpB_pool", bufs=2) as tmpBp, \
                 tc.tile_pool(name="recp", bufs=3) as recp, \
                 tc.tile_pool(name="rbbp", bufs=3) as rbbp, \
                 tc.tile_pool(name="epool", bufs=3) as epool, \
                 tc.tile_pool(name="outp", bufs=4) as outp, \
                 tc.tile_pool(name="qpsum", bufs=1, space="PSUM") as qpsum, \
                 tc.tile_pool(name="spsum", bufs=2, space="PSUM") as spsum, \
                 tc.tile_pool(name="opsum", bufs=2, space="PSUM") as opsum, \
                 tc.tile_pool(name="rbpsum", bufs=1, space="PSUM") as rbpsum:
                hidT = [hidTp.tile([P, S], bf16, name=f"hidT{i}") for i in range(8)]
                w_sb = [wpool.tile([P, 768], bf16, name=f"w{i}") for i in range(8)]
                for i in range(8):
                    nc.sync.dma_start(out=w_sb[i][:], in_=wqkv[i * P : (i + 1) * P, :])
                for dt_ in range(8):
                    nc.sync.dma_start_transpose(
                        out=hidT[dt_][:], in_=hid[:, dt_ * P : (dt_ + 1) * P]
                    )

                def qkv_chunk(ct, qc):
                    # ct: 0,1 = Q pairs; 2,3 = K pairs (wqkv col tiles 0-3)
                    ps = qpsum.tile([P, 512], f32, tag="qkvps")
                    for dt_ in range(8):
                        nc.tensor.matmul(
                            ps[:],
                            lhsT=w_sb[dt_][:, ct * P : (ct + 1) * P],
                            rhs=hidT[dt_][:, qc * 512 : (qc + 1) * 512],
                            start=(dt_ == 0),
                            stop=(dt_ == 7),
                        )
                    nc.vector.tensor_copy(
                        qkvT[ct][:, qc * 512 : (qc + 1) * 512], ps[:]
                    )

                def v_tile(sk):
                    # V seq-major: [sk, 4 heads * 64] = hidT_tile.T @ wv
                    vt = qpsum.tile([P, 512], f32, tag="qkvps")
                    for dt_ in range(8):
                        nc.tensor.matmul(
                            vt[:, 0:256],
                            lhsT=hidT[dt_][:, sk * P : (sk + 1) * P],
                            rhs=w_sb[dt_][:, 512:768],
                            start=(dt_ == 0),
                            stop=(dt_ == 7),
                        )
                    for h in range(NH):
                        base = (h * 16 + sk) * 65
                        nc.vector.tensor_copy(
                            vaug[:, base : base + HD], vt[:, h * HD : (h + 1) * HD]
                        )

                def proj_st(st):
                    ot = outp.tile([P, D], bf16, tag="ot")
                    for ec in range(2):
                        pp = qpsum.tile([P, 512], f32, tag="qkvps")
                        for hp2 in range(2):
                            nc.tensor.matmul(
                                pp[:],
                                lhsT=obar[hp2][:, st * P : (st + 1) * P],
                                rhs=wp_sb[hp2][:, ec * 512 : (ec + 1) * 512],
                                start=(hp2 == 0),
                                stop=(hp2 == 1),
                            )
                        nc.vector.tensor_copy(ot[:, ec * 512 : (ec + 1) * 512], pp[:])
                    nc.sync.dma_start(out=out[st * P : (st + 1) * P, :], in_=ot[:])

                def attn_pair(hp, inject_kt=None, after_qc=None):
                    qT = qkvT[0 + hp]
                    kT = qkvT[2 + hp]
                    tmpB = tmpBp.tile([HD, S], bf16, tag="tmpB")
                    for qc in range(4):
                        q0 = qc * 512
                        o_ps = [
                            opsum.tile([65, 512], f32, tag="ops", name=f"ops{jj}")
                            for jj in range(2)
                        ]
                        for kt in range(16):
                            if inject_kt is not None:
                                inject_kt(qc, kt)
                            sp = spsum.tile([P, 1024], f32, tag="sp")
                            for j in range(2):
                                par = HD * j
                                nc.tensor.matmul(
                                    sp[:, j * 512 : (j + 1) * 512],
                                    lhsT=kT[par : par + HD, kt * P : (kt + 1) * P],
                                    rhs=qT[par : par + HD, q0 : q0 + 512],
                                    start=True,
                                    stop=True,
                                )
                            eb = epool.tile([P, 1024], bf16, tag="eb")
                            nc.scalar.activation(eb[:], sp[:], AF.Exp)
                            for j in range(2):
                                h = 2 * hp + j
                                base = (h * 16 + kt) * 65
                                nc.tensor.matmul(
                                    o_ps[j][:],
                                    lhsT=vaug[:, base : base + 65],
                                    rhs=eb[:, j * 512 : (j + 1) * 512],
                                    start=(kt == 0),
                                    stop=(kt == 15),
                                )
                        for j in range(2):
                            ou = recp.tile([65, 512], f32, tag="ou")
                            nc.vector.tensor_copy(ou[:], o_ps[j][:])
                            rec = recp.tile([65, 512], f32, tag="rec")
                            rec_r = recp.tile([65, 512], f32r, tag="recr")
                            with nc.allow_low_precision(
                                reason="approx recip of softmax denom"
                            ):
                                nc.vector.reciprocal(
                                    rec[64:65, :], ou[64:65, :]
                                )
                                nc.vector.tensor_copy(
                                    rec_r[64:65, :], rec[64:65, :]
                                )
                            rb_ps = rbpsum.tile([65, 512], f32, tag="rb")
                            nc.tensor.matmul(
                                rb_ps[0:HD, :],
                                lhsT=ones_r[64:65, 0:HD],
                                rhs=rec_r[64:65, :],
                                start=True,
                                stop=True,
                            )
                            rbb = rbbp.tile([HD, 512], f32, tag="rbb")
                            nc.vector.tensor_copy(rbb[:], rb_ps[0:HD, :])
                            dst = obar[hp] if j == 0 else tmpB
                            with nc.allow_low_precision(
                                reason="softmax normalize, bf16 out"
                            ):
                                nc.vector.tensor_tensor(
                                    out=dst[0:HD, q0 : q0 + 512],
                                    in0=ou[0:HD, :],
                                    in1=rbb[:],
                                    op=ALU.mult,
                                )
                        # shift head B rows of this q chunk into the pair tile
                        nc.sync.dma_start(
                            out=obar[hp][HD:P, q0 : q0 + 512],
                            in_=tmpB[:, q0 : q0 + 512],
                        )
                        if after_qc is not None:
                            after_qc(qc)

                for qc in range(4):
                    qkv_chunk(0, qc)  # Q pair0
                    qkv_chunk(2, qc)  # K pair0

                _pend = [(1, 0), (3, 0), (1, 1), (3, 1), (1, 2), (3, 2), (1, 3), (3, 3)]

                def _inj0(qc, kt):
                    if qc == 0:
                        v_tile(kt)
                    elif kt % 6 == 2 and _pend:
                        ct, q2 = _pend.pop(0)
                        qkv_chunk(ct, q2)

                attn_pair(0, inject_kt=_inj0)
                while _pend:
                    ct, q2 = _pend.pop(0)
                    qkv_chunk(ct, q2)

                def _after1(qc):
                    for st in range(4 * qc, 4 * qc + 4):
                        proj_st(st)

                attn_pair(1, after_qc=_after1)

    nc.compile()
    return nc


def _get_nc():
    if "nc" not in _CACHE:
        _CACHE["nc"] = _build_program()
    return _CACHE["nc"]


def _shard_inputs(hidden_states, w_attn, w_proj):
    import ml_dtypes

    bf16 = ml_dtypes.bfloat16
    scale = 1.0 / np.sqrt(np.float32(HD))
    hid_b = [np.ascontiguousarray(hidden_states[b].astype(bf16)) for b in range(2)]
    in_maps = []
    for c in range(N_CORES):
        b, g = divmod(c, 4)
        cs = slice(g * NH * HD, (g + 1) * NH * HD)
        wq = w_attn[:, 0:D][:, cs] * scale
        wk = w_attn[:, D : 2 * D][:, cs]
        wv = w_attn[:, 2 * D : 3 * D][:, cs]
        in_maps.append(
            {
                "hid": hid_b[b],
                "wqkv": np.ascontiguousarray(
                    np.concatenate([wq, wk, wv], axis=1).astype(bf16)
                ),
                "wp": np.ascontiguousarray(w_proj[cs, :].astype(bf16)),
            }
        )
    return in_maps


def run(hidden_states, w_attn, w_proj, trace=False):
    from concourse.bass_utils import run_bass_kernel_spmd

    nc = _get_nc()
    in_maps = _shard_inputs(hidden_states, w_attn, w_proj)
    res = run_bass_kernel_spmd(nc, in_maps, list(range(N_CORES)), trace=trace)
    parts = [res.results[c]["out"].astype(np.float32) for c in range(N_CORES)]
    out = np.stack(
        [
            parts[0] + parts[1] + parts[2] + parts[3],
            parts[4] + parts[5] + parts[6] + parts[7],
        ]
    ).astype(np.float32)
    return out, res


def kernel(hidden_states, w_attn, w_proj):
    out, _ = run(np.asarray(hidden_states), np.asarray(w_attn), np.asarray(w_proj))
    return out


def check_partials(parts, expected, inputs):
    """simtest helper: parts = {core: {'out': arr}}. Checks each simulated
    core's partial against the reference partial for that (batch, head group).
    """
    import jax
    import jax.numpy as jnp

    hs = inputs["hidden_states"]
    w_attn = inputs["w_attn"]
    w_proj = inputs["w_proj"]
    lines = []
    for c, outs in parts.items():
        b, g = divmod(c, 4)
        cs = slice(g * NH * HD, (g + 1) * NH * HD)
        x = jnp.asarray(hs[b])  # [S, D]
        q = x @ jnp.asarray(w_attn[:, 0:D][:, cs]) / np.sqrt(np.float32(HD))
        k = x @ jnp.asarray(w_attn[:, D : 2 * D][:, cs])
        v = x @ jnp.asarray(w_attn[:, 2 * D : 3 * D][:, cs])
        acc = jnp.zeros((S, D), dtype=jnp.float32)
        for h in range(NH):
            sl = slice(h * HD, (h + 1) * HD)
            sc = q[:, sl] @ k[:, sl].T
            pr = jax.nn.softmax(sc, axis=-1)
            acc = acc + (pr @ v[:, sl]) @ jnp.asarray(w_proj[cs, :][sl, :])
        ref = np.asarray(acc)
        got = np.asarray(outs["out"]).astype(np.float64)
        denom = np.sqrt((ref.astype(np.float64) ** 2).mean())
        rms = np.sqrt(((got - ref) ** 2).mean()) / denom
        lines.append(f"core {c}: partial rms rel err = {rms:.3e}")
    return "\n".join(lines)
